# revision 37
# baseline (speedup 1.0000x reference)
"""TRN2 Bass kernel for nn_ClassAttention (1x1 conv + BN + ReLU + windowed attention).

kernel(**inputs) takes FULL inputs, returns the FULL output [4,256,256,256] f32.
Shards data-parallel over (batch, image-row-half) across 8 NeuronCores, runs a
Bass/Tile SPMD program via run_bass_kernel_spmd, and unshards on the host.

Per-core shard (core = (b, rh) = (core//2, core%2)):
  x_sh   [128, 16hh, 4096]    x[b,:,128rh:+128,:] fp8-e3m4, pair-major layout
                              (pw, half, win, r1, r2) so each pair block holds
                              both channel halves contiguously
  at_sh  [16hh, 128, 16384]   attn fp8-e3m4, per-(win,nh,q)-row scaled on host
                              (amax -> 15.0); partition = 64*win+k, free =
                              (pair, nh, q); dequant scales applied in decode
  w_prep [256c, 256o]         (w_conv * inv_std[:,None]).T fp16 (BN folded)
  bias   [128, 1024]          (beta - mean*inv_std) broadcast over partitions
  out    [16hh, 128p, 4096]   raw staging dump fp16; host decodes

Per group of GB=4 window-pairs (pixels on psum partitions), software-pipelined
with LOOKAHEAD=2 so the cross-engine chain conv->ADD(DVE)->RELU(ACT)->attn(PE)
hides across two pipeline iterations (keeps the PE fed and the HAM clock gate
warm). Each iteration emits finish(g-2) BEFORE stage(g) so psum evacs are
never stuck behind fresh ADDs in the DVE FIFO:
  finish(g-2):
    attn (PE): per (pair, head): out[32,64] = V[:,32nh:+32].T @ At[:,64nh:+64]
               fp16 stationary (block-diag V) x fp8-e3m4 moving, K=128, N=64,
               tile_position=(0, 32*(nh%4)) -> 4 column-strips packed
    evac (DVE): attn psum -> staging fp16
    store: 0.5 MiB per group via the scalar hwdge ring
  stage(g):
    conv (PE): psum[128pix=(win,r1,r2), 256ch] = x_pair.T @ w_prep
               fp8 stationary x fp16 moving, 2 matmuls (K=128 halves), N=256
    bias (DVE): tv = psum + bias_tile
    relu (ACT): block-diagonal V [128, (nh,win,d)]: diag cells = relu(tv),
                off-diag cells stay zero (zeroed once at start)

Quantization (harness gate rel_err < 2e-2; this kernel measures ~1.6e-2):
attn + x in fp8-e3m4 (4 mantissa bits), attn rows scaled to amax=15 with the
scales folded into the host-side decode; V/w fp16; all matmul accum fp32.
"""

import numpy as np
import ml_dtypes
from contextlib import ExitStack

import concourse.bacc as bacc
import concourse.tile as tile
import concourse.mybir as mybir
from concourse.bass_utils import run_bass_kernel_spmd

F32 = mybir.dt.float32
F16 = mybir.dt.float16
F8E3 = mybir.dt.float8e3
RELU = mybir.ActivationFunctionType.Relu

EPS = 1e-5
NCORES = 8

_cached_nc = None


def _build_program(n_vbd=4, at_bufs=3, LOOKAHEAD=2):
    nc = bacc.Bacc("TRN2", target_bir_lowering=False, debug=False)

    x_d = nc.dram_tensor("x_sh", [128, 16, 4096], F8E3, kind="ExternalInput")
    at_d = nc.dram_tensor("at_sh", [16, 128, 16384], F8E3, kind="ExternalInput")
    wc_d = nc.dram_tensor("w_prep", [256, 256], F16, kind="ExternalInput")
    b_d = nc.dram_tensor("bias", [128, 1024], F32, kind="ExternalInput")
    out_d = nc.dram_tensor("out_sh", [16, 128, 4096], F16, kind="ExternalOutput")

    GB = 4                   # pairs per elementwise batch group

    with tile.TileContext(nc) as tc, ExitStack() as ctx:
        const = ctx.enter_context(tc.tile_pool(name="const", bufs=1))
        xp = ctx.enter_context(tc.tile_pool(name="xp", bufs=3))
        atp = ctx.enter_context(tc.tile_pool(name="atp", bufs=at_bufs))
        vbdp = ctx.enter_context(tc.tile_pool(name="vbdp", bufs=1))
        tvp = ctx.enter_context(tc.tile_pool(name="tvp", bufs=4))
        stp = ctx.enter_context(tc.tile_pool(name="stp", bufs=3))
        pscp = ctx.enter_context(tc.tile_pool(name="pscp", bufs=2, space="PSUM"))
        psap = ctx.enter_context(tc.tile_pool(name="psap", bufs=2, space="PSUM"))

        # const loads go on the scalar HWDGE ring so the sync ring's FIFO
        # starts with the bulk at/x loads immediately
        w0 = const.tile([128, 256], F16, name="w0")
        w1 = const.tile([128, 256], F16, name="w1")
        nc.scalar.dma_start(out=w0, in_=wc_d[0:128, :])
        nc.scalar.dma_start(out=w1, in_=wc_d[128:256, :])
        bias = const.tile([128, 1024], F32, name="bias_t")
        nc.scalar.dma_start(out=bias, in_=b_d[:, :])

        # Block-diagonal V tiles for GB pairs each: columns =
        # (pair GB, nh 16, win 2, d 16). Zeroed once; the relu writes only the
        # diagonal cells (win0 -> rows 0:64 of win-0 columns, win1 -> rows
        # 64:128 of win-1 columns), so the zeros persist across reuse and each
        # V[:, 512p+32nh:+32] is exactly block-diag(V0, V1).
        # lazily zeroed: only the first two upfront so the DVE queue reaches
        # the first ADD quickly; the rest are zeroed one per iteration below
        vbd = [vbdp.tile([128, 512 * GB], F16, tag=f"vbd{i}", name=f"vbd{i}")
               for i in range(n_vbd)]
        nc.vector.memset(vbd[0], 0.0)
        nc.vector.memset(vbd[1], 0.0)
        vbd_zeroed = 2

        # HAM warm-up: the PE clock gate defaults to 1.2 GHz and needs ~3.4us
        # of sustained matmul activity to release to 2.4 GHz. The PE would
        # otherwise idle ~8us waiting for the first x tile, so burn that time
        # on dummy matmuls (w0 against a zeroed tile into a scratch psum
        # slot) and start the real conv already warm.
        warm = const.tile([128, 128], F16, name="warm")
        nc.vector.memset(warm, 0.0)
        wps = psap.tile([128, 256], F32, tag="pa4", name="warm_ps")
        for i in range(28):
            nc.tensor.matmul(wps, warm, w0, start=True, stop=True)

        # 3-stage software pipeline with LOOKAHEAD=2: for group g issue
        # conv(g)+add(g)+relu(g), then finish(g-2) = attn+evac+store. The
        # cross-engine latency chain conv->ADD(DVE)->RELU(ACT)->attn(PE)
        # (~3.5us) then hides across two pipeline cycles, so the PE never
        # starves (which would also re-throttle the HAM clock gate).
        pending = []
        vbd_i = 0
        evac_i = 0

        def iteration(cur, prev):
            """Emit one pipeline iteration: conv+add+relu for `cur`, attn+
            evac+store for `prev` (LOOKAHEAD iterations older), with the conv
            pair-blocks INTERLEAVED between attn pair-blocks. Each ~110ns
            conv matmul lets the sequencer rebuild PE-queue lead so the attn
            matmuls run queue-fed (multiple column-strips streaming) instead
            of dispatch-dribble."""
            nonlocal vbd_i, evac_i, vbd_zeroed
            ps4 = tv4 = V4c = None
            if prev is not None:
                V4, at_p, st_p, hh_p, bg_p = prev
                pa4 = psap.tile([128, 256 * GB], F32, tag="pa4",
                                name=f"pa4_{hh_p}_{bg_p}")
                for p in range(GB):
                    ploc = GB * bg_p + p       # pair index in at tile
                    for j in range(4):
                        for quad in range(4):
                            nh = 4 * j + quad
                            nc.tensor.matmul(
                                pa4[32 * quad:32 * quad + 32,
                                    256 * p + 64 * j:256 * p + 64 * j + 64],
                                V4[:, 512 * p + 32 * nh:
                                   512 * p + 32 * nh + 32],
                                at_p[:, 1024 * ploc + 64 * nh:
                                     1024 * ploc + 64 * nh + 64],
                                start=True, stop=True,
                                tile_position=(0, 32 * quad))
                osl_p = slice(1024 * bg_p, 1024 * bg_p + 1024)
                nc.vector.tensor_copy(st_p[:, osl_p], pa4)
                evac_i += 1
                nc.scalar.dma_start(out=out_d[hh_p, :, osl_p],
                                    in_=st_p[:, osl_p])
            if cur is not None:
                xt, at, st, hh, bg = cur
                if vbd_zeroed < n_vbd:
                    nc.vector.memset(vbd[vbd_zeroed], 0.0)
                    vbd_zeroed += 1
                ps4 = pscp.tile([128, 256 * GB], F32, tag="ps4",
                                name=f"ps4_{hh}_{bg}")
                for p in range(GB):
                    p16 = GB * bg + p          # pair index in hh
                    xsl0 = slice(256 * p16, 256 * p16 + 128)
                    xsl1 = slice(256 * p16 + 128, 256 * p16 + 256)
                    osl = slice(256 * p, 256 * p + 256)
                    nc.tensor.matmul(ps4[:, osl], xt[:, xsl0], w0,
                                     start=True, stop=False)
                    nc.tensor.matmul(ps4[:, osl], xt[:, xsl1], w1,
                                     start=False, stop=True)
                tv4 = tvp.tile([128, 256 * GB], F16, tag="tv4",
                               name=f"tv4_{hh}_{bg}")
                nc.vector.tensor_add(tv4, ps4, bias)
                V4c = vbd[vbd_i % n_vbd]
                vbd_i += 1
                Vr = V4c.rearrange("pt (p nh two d) -> pt p nh two d",
                                   p=GB, nh=16, two=2, d=16)
                tvr = tv4.rearrange("pt (p a b) -> pt p a b", p=GB, a=16)
                nc.scalar.activation(Vr[0:64, :, :, 0, :], tvr[0:64], RELU)
                nc.scalar.activation(Vr[64:128, :, :, 1, :], tvr[64:128],
                                     RELU)
                return (V4c, at, st, hh, bg)
            return None

        for hh in range(16):
            xt = xp.tile([128, 4096], F8E3, tag="xt", name=f"xt_{hh}")
            if hh == 0:
                # pair-major x layout: quarter q holds exactly group q's
                # pairs, so conv(0,0) can start after the first 256 KiB
                for q in range(4):
                    nc.sync.dma_start(out=xt[:, 1024 * q:1024 * q + 1024],
                                      in_=x_d[:, hh, 1024 * q:1024 * q + 1024])
            else:
                nc.sync.dma_start(out=xt, in_=x_d[:, hh, :])
            # all 16 heads arrive fp8-e3m4 (per-row scaled on host; dequant
            # on host during decode); the PE consumes fp8 moving operands
            # directly -- no widen pass. Two half-loads so the first groups
            # of the row do not wait on the full 2 MiB tile.
            at = atp.tile([128, 16384], F8E3, tag="at", name=f"at_{hh}")
            nc.sync.dma_start(out=at[:, 0:8192], in_=at_d[hh, :, 0:8192])
            nc.sync.dma_start(out=at[:, 8192:16384],
                              in_=at_d[hh, :, 8192:16384])
            st = stp.tile([128, 4096], F16, tag="st", name=f"st_{hh}")

            for bg in range(4):
                prev = pending.pop(0) if len(pending) >= LOOKAHEAD else None
                pending.append(iteration((xt, at, st, hh, bg), prev))
        for state in pending:
            iteration(None, state)

    nc.compile()
    return nc


def _shard_inputs(x, attn_i, w_conv, bn_gamma, bn_beta, bn_mean, bn_var):
    inv_std = (bn_gamma / np.sqrt(bn_var + np.float32(EPS))).astype(np.float32)
    shift = (bn_beta - bn_mean * inv_std).astype(np.float32)
    bias_tile = np.ascontiguousarray(
        np.broadcast_to(np.tile(shift, 4)[None, :], (128, 1024))
    ).astype(np.float32)
    w_prep = np.ascontiguousarray(
        (w_conv * inv_std[:, None]).T).astype(np.float16)
    x16 = x.astype(np.float32)
    # all 16 heads: fp8 e3m4 with per-(win,head,q)-row scales mapping the
    # row amax to 15.0 (e3m4 max normal is 15.5); dequant on host in decode
    amax = np.maximum(np.abs(attn_i).max(axis=3, keepdims=True), 1e-9)
    s_a = (amax / np.float32(15.0)).astype(np.float32)  # [4096, 16, 64, 1]
    a8 = (attn_i / s_a).astype(ml_dtypes.float8_e3m4)
    in_maps = []
    scales = []
    for core in range(NCORES):
        b, rh = core // 2, core % 2
        x_sh = x16[b, :, 128 * rh:128 * rh + 128, :]
        # pair-major layout: [cl, hh, (pw, half, win, r1, r2)] -- each pair's
        # 256-col block holds both channel halves, so a quarter of the row
        # covers one whole conv group
        x_sh = np.ascontiguousarray(
            x_sh.reshape(2, 128, 16, 8, 16, 2, 8).transpose(1, 2, 4, 0, 5, 3, 6)
        ).reshape(128, 16, 4096).astype(ml_dtypes.float8_e3m4)
        sl = slice(1024 * b + 512 * rh, 1024 * b + 512 * rh + 512)

        def prep(a):  # [512, 16, 64, 64] -> [16, 128, 16384], pair-transposed
            p = a.reshape(256, 2, 16, 64, 64).transpose(0, 1, 4, 2, 3) \
                .reshape(16, 16, 128, 1024)
            return np.ascontiguousarray(
                p.transpose(0, 2, 1, 3)).reshape(16, 128, 16384)

        scales.append(s_a[sl])
        in_maps.append(dict(x_sh=x_sh, at_sh=prep(a8[sl]),
                            w_prep=w_prep, bias=bias_tile))
    return in_maps, scales


def _unshard_output(results, scales):
    out = np.empty((4, 256, 256, 256), np.float32)
    for core in range(NCORES):
        b, rh = core // 2, core % 2
        raw = np.asarray(results[core]["out_sh"], np.float32)  # [16, 128, 4096]
        # partition = (quad4, win2, d16); f = pw*256 + j*64 + ws1*8 + ws2
        r = raw.reshape(16, 4, 2, 16, 16, 4, 8, 8)  # hh,quad,win,d,pw,j,ws1,ws2
        # dequant all heads: scales[core] [512=(hh,pw,win), 16nh, 64q, 1]
        s = scales[core].reshape(16, 16, 2, 4, 4, 8, 8)  # hh,pw,win,j,quad,ws1,ws2
        # -> [hh, quad, win, 1(d), pw, j, ws1, ws2]
        s = s.transpose(0, 4, 2, 1, 3, 5, 6)[:, :, :, None]
        r = r * s
        # ch = 16*(4j+quad)+d ; h = 8hh+ws1 ; w = 16pw+8win+ws2
        oc = r.transpose(5, 1, 3, 0, 6, 4, 2, 7).reshape(256, 128, 256)
        out[b, :, 128 * rh:128 * rh + 128, :] = oc
    return out


def get_program():
    global _cached_nc
    if _cached_nc is None:
        _cached_nc = _build_program()
    return _cached_nc


def run_sharded(in_maps, trace=False, **kwargs):
    nc = get_program()
    return run_bass_kernel_spmd(nc, in_maps, list(range(NCORES)),
                                trace=trace, **kwargs)


def kernel(x, attn_i, w_conv, bn_gamma, bn_beta, bn_mean, bn_var):
    x = np.asarray(x, dtype=np.float32)
    attn_i = np.asarray(attn_i, dtype=np.float32)
    w_conv = np.asarray(w_conv, dtype=np.float32)
    bn_gamma = np.asarray(bn_gamma, dtype=np.float32)
    bn_beta = np.asarray(bn_beta, dtype=np.float32)
    bn_mean = np.asarray(bn_mean, dtype=np.float32)
    bn_var = np.asarray(bn_var, dtype=np.float32)
    in_maps, scales = _shard_inputs(x, attn_i, w_conv, bn_gamma, bn_beta,
                                    bn_mean, bn_var)
    res = run_sharded(in_maps)
    return _unshard_output(res.results, scales)



# revision 38
# speedup vs baseline: 1.0866x; 1.0866x over previous
"""TRN2 Bass kernel for nn_ClassAttention (1x1 conv + BN + ReLU + windowed attention).

kernel(**inputs) takes FULL inputs, returns the FULL output [4,256,256,256] f32.
Shards data-parallel over (batch, image-row-half) across 8 NeuronCores, runs a
Bass/Tile SPMD program via run_bass_kernel_spmd, and unshards on the host.

Per-core shard (core = (b, rh) = (core//2, core%2)):
  x_sh   [128, 16hh, 4096]    x[b,:,128rh:+128,:] fp8-e3m4, pair-major layout
                              (pw, half, win, r1, r2) so each pair block holds
                              both channel halves contiguously
  at_sh  [16hh, 128, 16384]   attn fp8-e3m4, per-(win,nh,q)-row scaled on host
                              (amax -> 15.0); partition = 64*win+k, free =
                              (pair, nh, q); dequant scales applied in decode
  w_prep [256c, 256o]         (w_conv * inv_std[:,None]).T fp16 (BN folded)
  bias   [128, 1024]          (beta - mean*inv_std) broadcast over partitions
  out    [16hh, 128p, 4096]   raw staging dump fp16; host decodes

Per group of GB=4 window-pairs (pixels on psum partitions), software-pipelined
with LOOKAHEAD=2 so the cross-engine chain conv->ADD(DVE)->RELU(ACT)->attn(PE)
hides across two pipeline iterations (keeps the PE fed and the HAM clock gate
warm). Each iteration emits finish(g-2) BEFORE stage(g) so psum evacs are
never stuck behind fresh ADDs in the DVE FIFO:
  finish(g-2):
    attn (PE): per (pair, head): out[32,64] = V[:,32nh:+32].T @ At[:,64nh:+64]
               fp16 stationary (block-diag V) x fp8-e3m4 moving, K=128, N=64,
               tile_position=(0, 32*(nh%4)) -> 4 column-strips packed
    evac (DVE): attn psum -> staging fp16
    store: 0.5 MiB per group via the scalar hwdge ring
  stage(g):
    conv (PE): psum[128pix=(win,r1,r2), 256ch] = x_pair.T @ w_prep
               fp8 stationary x fp16 moving, 2 matmuls (K=128 halves), N=256
    bias (DVE): tv = psum + bias_tile
    relu (ACT): block-diagonal V [128, (nh,win,d)]: diag cells = relu(tv),
                off-diag cells stay zero (zeroed once at start)

Quantization (harness gate rel_err < 2e-2; this kernel measures ~1.6e-2):
attn + x in fp8-e3m4 (4 mantissa bits), attn rows scaled to amax=15 with the
scales folded into the host-side decode; V/w fp16; all matmul accum fp32.
"""

import numpy as np
import ml_dtypes
from contextlib import ExitStack

import concourse.bacc as bacc
import concourse.tile as tile
import concourse.mybir as mybir
from concourse.bass_utils import run_bass_kernel_spmd

F32 = mybir.dt.float32
F16 = mybir.dt.float16
F8E3 = mybir.dt.float8e3
RELU = mybir.ActivationFunctionType.Relu

EPS = 1e-5
NCORES = 8

_cached_nc = None


def _build_program(n_vbd=4, at_bufs=3, LOOKAHEAD=2):
    nc = bacc.Bacc("TRN2", target_bir_lowering=False, debug=False)

    x_d = nc.dram_tensor("x_sh", [128, 16, 4096], F8E3, kind="ExternalInput")
    at_d = nc.dram_tensor("at_sh", [16, 128, 16384], F8E3, kind="ExternalInput")
    wc_d = nc.dram_tensor("w_prep", [256, 256], F16, kind="ExternalInput")
    b_d = nc.dram_tensor("bias", [128, 1024], F32, kind="ExternalInput")
    out_d = nc.dram_tensor("out_sh", [16, 128, 4096], F16, kind="ExternalOutput")

    GB = 4                   # pairs per elementwise batch group

    with tile.TileContext(nc) as tc, ExitStack() as ctx:
        const = ctx.enter_context(tc.tile_pool(name="const", bufs=1))
        xp = ctx.enter_context(tc.tile_pool(name="xp", bufs=3))
        atp = ctx.enter_context(tc.tile_pool(name="atp", bufs=at_bufs))
        vbdp = ctx.enter_context(tc.tile_pool(name="vbdp", bufs=1))
        tvp = ctx.enter_context(tc.tile_pool(name="tvp", bufs=4))
        stp = ctx.enter_context(tc.tile_pool(name="stp", bufs=3))
        pscp = ctx.enter_context(tc.tile_pool(name="pscp", bufs=2, space="PSUM"))
        psap = ctx.enter_context(tc.tile_pool(name="psap", bufs=2, space="PSUM"))

        # const loads go on the scalar HWDGE ring so the sync ring's FIFO
        # starts with the bulk at/x loads immediately
        w0 = const.tile([128, 256], F16, name="w0")
        w1 = const.tile([128, 256], F16, name="w1")
        nc.scalar.dma_start(out=w0, in_=wc_d[0:128, :])
        nc.scalar.dma_start(out=w1, in_=wc_d[128:256, :])
        bias = const.tile([128, 1024], F32, name="bias_t")
        nc.scalar.dma_start(out=bias, in_=b_d[:, :])

        # Block-diagonal V tiles for GB pairs each: columns =
        # (pair GB, nh 16, win 2, d 16). Zeroed once; the relu writes only the
        # diagonal cells (win0 -> rows 0:64 of win-0 columns, win1 -> rows
        # 64:128 of win-1 columns), so the zeros persist across reuse and each
        # V[:, 512p+32nh:+32] is exactly block-diag(V0, V1).
        # lazily zeroed: only the first two upfront so the DVE queue reaches
        # the first ADD quickly; the rest are zeroed one per iteration below
        vbd = [vbdp.tile([128, 512 * GB], F16, tag=f"vbd{i}", name=f"vbd{i}")
               for i in range(n_vbd)]
        nc.vector.memset(vbd[0], 0.0)
        nc.vector.memset(vbd[1], 0.0)
        vbd_zeroed = 2

        # 3-stage software pipeline with LOOKAHEAD=2: for group g issue
        # conv(g)+add(g)+relu(g), then finish(g-2) = attn+evac+store. The
        # cross-engine latency chain conv->ADD(DVE)->RELU(ACT)->attn(PE)
        # (~3.5us) then hides across two pipeline cycles, so the PE never
        # starves (which would also re-throttle the HAM clock gate).
        pending = []
        vbd_i = 0
        evac_i = 0

        def iteration(cur, prev):
            """Emit one pipeline iteration: conv+add+relu for `cur`, attn+
            evac+store for `prev` (LOOKAHEAD iterations older), with the conv
            pair-blocks INTERLEAVED between attn pair-blocks. Each ~110ns
            conv matmul lets the sequencer rebuild PE-queue lead so the attn
            matmuls run queue-fed (multiple column-strips streaming) instead
            of dispatch-dribble."""
            nonlocal vbd_i, evac_i, vbd_zeroed
            ps4 = tv4 = V4c = None
            if prev is not None:
                V4, at_p, st_p, hh_p, bg_p = prev
                pa4 = psap.tile([128, 256 * GB], F32, tag="pa4",
                                name=f"pa4_{hh_p}_{bg_p}")
                for p in range(GB):
                    ploc = GB * bg_p + p       # pair index in at tile
                    for j in range(4):
                        for quad in range(4):
                            nh = 4 * j + quad
                            nc.tensor.matmul(
                                pa4[32 * quad:32 * quad + 32,
                                    256 * p + 64 * j:256 * p + 64 * j + 64],
                                V4[:, 512 * p + 32 * nh:
                                   512 * p + 32 * nh + 32],
                                at_p[:, 1024 * ploc + 64 * nh:
                                     1024 * ploc + 64 * nh + 64],
                                start=True, stop=True,
                                tile_position=(0, 32 * quad))
                osl_p = slice(1024 * bg_p, 1024 * bg_p + 1024)
                nc.vector.tensor_copy(st_p[:, osl_p], pa4)
                evac_i += 1
                nc.scalar.dma_start(out=out_d[hh_p, :, osl_p],
                                    in_=st_p[:, osl_p])
            if cur is not None:
                xt, at, st, hh, bg = cur
                if vbd_zeroed < n_vbd:
                    nc.vector.memset(vbd[vbd_zeroed], 0.0)
                    vbd_zeroed += 1
                ps4 = pscp.tile([128, 256 * GB], F32, tag="ps4",
                                name=f"ps4_{hh}_{bg}")
                for p in range(GB):
                    p16 = GB * bg + p          # pair index in hh
                    xsl0 = slice(256 * p16, 256 * p16 + 128)
                    xsl1 = slice(256 * p16 + 128, 256 * p16 + 256)
                    osl = slice(256 * p, 256 * p + 256)
                    nc.tensor.matmul(ps4[:, osl], xt[:, xsl0], w0,
                                     start=True, stop=False)
                    nc.tensor.matmul(ps4[:, osl], xt[:, xsl1], w1,
                                     start=False, stop=True)
                tv4 = tvp.tile([128, 256 * GB], F16, tag="tv4",
                               name=f"tv4_{hh}_{bg}")
                nc.vector.tensor_add(tv4, ps4, bias)
                V4c = vbd[vbd_i % n_vbd]
                vbd_i += 1
                Vr = V4c.rearrange("pt (p nh two d) -> pt p nh two d",
                                   p=GB, nh=16, two=2, d=16)
                tvr = tv4.rearrange("pt (p a b) -> pt p a b", p=GB, a=16)
                nc.scalar.activation(Vr[0:64, :, :, 0, :], tvr[0:64], RELU)
                nc.scalar.activation(Vr[64:128, :, :, 1, :], tvr[64:128],
                                     RELU)
                return (V4c, at, st, hh, bg)
            return None

        for hh in range(16):
            xt = xp.tile([128, 4096], F8E3, tag="xt", name=f"xt_{hh}")
            if hh == 0:
                # pair-major x layout: quarter q holds exactly group q's
                # pairs, so conv(0,0) can start after the first 256 KiB
                for q in range(4):
                    nc.sync.dma_start(out=xt[:, 1024 * q:1024 * q + 1024],
                                      in_=x_d[:, hh, 1024 * q:1024 * q + 1024])
            else:
                nc.sync.dma_start(out=xt, in_=x_d[:, hh, :])
            # all 16 heads arrive fp8-e3m4 (per-row scaled on host; dequant
            # on host during decode); the PE consumes fp8 moving operands
            # directly -- no widen pass. Two half-loads so the first groups
            # of the row do not wait on the full 2 MiB tile.
            at = atp.tile([128, 16384], F8E3, tag="at", name=f"at_{hh}")
            nc.sync.dma_start(out=at[:, 0:8192], in_=at_d[hh, :, 0:8192])
            nc.sync.dma_start(out=at[:, 8192:16384],
                              in_=at_d[hh, :, 8192:16384])
            st = stp.tile([128, 4096], F16, tag="st", name=f"st_{hh}")

            for bg in range(4):
                prev = pending.pop(0) if len(pending) >= LOOKAHEAD else None
                pending.append(iteration((xt, at, st, hh, bg), prev))
        for state in pending:
            iteration(None, state)

    nc.compile()
    return nc


def _shard_inputs(x, attn_i, w_conv, bn_gamma, bn_beta, bn_mean, bn_var):
    inv_std = (bn_gamma / np.sqrt(bn_var + np.float32(EPS))).astype(np.float32)
    shift = (bn_beta - bn_mean * inv_std).astype(np.float32)
    bias_tile = np.ascontiguousarray(
        np.broadcast_to(np.tile(shift, 4)[None, :], (128, 1024))
    ).astype(np.float32)
    w_prep = np.ascontiguousarray(
        (w_conv * inv_std[:, None]).T).astype(np.float16)
    x16 = x.astype(np.float32)
    # all 16 heads: fp8 e3m4 with per-(win,head,q)-row scales mapping the
    # row amax to 15.0 (e3m4 max normal is 15.5); dequant on host in decode
    amax = np.maximum(np.abs(attn_i).max(axis=3, keepdims=True), 1e-9)
    s_a = (amax / np.float32(15.0)).astype(np.float32)  # [4096, 16, 64, 1]
    a8 = (attn_i / s_a).astype(ml_dtypes.float8_e3m4)
    in_maps = []
    scales = []
    for core in range(NCORES):
        b, rh = core // 2, core % 2
        x_sh = x16[b, :, 128 * rh:128 * rh + 128, :]
        # pair-major layout: [cl, hh, (pw, half, win, r1, r2)] -- each pair's
        # 256-col block holds both channel halves, so a quarter of the row
        # covers one whole conv group
        x_sh = np.ascontiguousarray(
            x_sh.reshape(2, 128, 16, 8, 16, 2, 8).transpose(1, 2, 4, 0, 5, 3, 6)
        ).reshape(128, 16, 4096).astype(ml_dtypes.float8_e3m4)
        sl = slice(1024 * b + 512 * rh, 1024 * b + 512 * rh + 512)

        def prep(a):  # [512, 16, 64, 64] -> [16, 128, 16384], pair-transposed
            p = a.reshape(256, 2, 16, 64, 64).transpose(0, 1, 4, 2, 3) \
                .reshape(16, 16, 128, 1024)
            return np.ascontiguousarray(
                p.transpose(0, 2, 1, 3)).reshape(16, 128, 16384)

        scales.append(s_a[sl])
        in_maps.append(dict(x_sh=x_sh, at_sh=prep(a8[sl]),
                            w_prep=w_prep, bias=bias_tile))
    return in_maps, scales


def _unshard_output(results, scales):
    out = np.empty((4, 256, 256, 256), np.float32)
    for core in range(NCORES):
        b, rh = core // 2, core % 2
        raw = np.asarray(results[core]["out_sh"], np.float32)  # [16, 128, 4096]
        # partition = (quad4, win2, d16); f = pw*256 + j*64 + ws1*8 + ws2
        r = raw.reshape(16, 4, 2, 16, 16, 4, 8, 8)  # hh,quad,win,d,pw,j,ws1,ws2
        # dequant all heads: scales[core] [512=(hh,pw,win), 16nh, 64q, 1]
        s = scales[core].reshape(16, 16, 2, 4, 4, 8, 8)  # hh,pw,win,j,quad,ws1,ws2
        # -> [hh, quad, win, 1(d), pw, j, ws1, ws2]
        s = s.transpose(0, 4, 2, 1, 3, 5, 6)[:, :, :, None]
        r = r * s
        # ch = 16*(4j+quad)+d ; h = 8hh+ws1 ; w = 16pw+8win+ws2
        oc = r.transpose(5, 1, 3, 0, 6, 4, 2, 7).reshape(256, 128, 256)
        out[b, :, 128 * rh:128 * rh + 128, :] = oc
    return out


def get_program():
    global _cached_nc
    if _cached_nc is None:
        _cached_nc = _build_program()
    return _cached_nc


def run_sharded(in_maps, trace=False, **kwargs):
    nc = get_program()
    return run_bass_kernel_spmd(nc, in_maps, list(range(NCORES)),
                                trace=trace, **kwargs)


def kernel(x, attn_i, w_conv, bn_gamma, bn_beta, bn_mean, bn_var):
    x = np.asarray(x, dtype=np.float32)
    attn_i = np.asarray(attn_i, dtype=np.float32)
    w_conv = np.asarray(w_conv, dtype=np.float32)
    bn_gamma = np.asarray(bn_gamma, dtype=np.float32)
    bn_beta = np.asarray(bn_beta, dtype=np.float32)
    bn_mean = np.asarray(bn_mean, dtype=np.float32)
    bn_var = np.asarray(bn_var, dtype=np.float32)
    in_maps, scales = _shard_inputs(x, attn_i, w_conv, bn_gamma, bn_beta,
                                    bn_mean, bn_var)
    res = run_sharded(in_maps)
    return _unshard_output(res.results, scales)



# revision 39
# speedup vs baseline: 1.1000x; 1.0123x over previous
"""TRN2 Bass kernel for nn_ClassAttention (1x1 conv + BN + ReLU + windowed attention).

kernel(**inputs) takes FULL inputs, returns the FULL output [4,256,256,256] f32.
Shards data-parallel over (batch, image-row-half) across 8 NeuronCores, runs a
Bass/Tile SPMD program via run_bass_kernel_spmd, and unshards on the host.

Per-core shard (core = (b, rh) = (core//2, core%2)):
  x_sh   [128, 16hh, 4096]    x[b,:,128rh:+128,:] fp8-e3m4, pair-major layout
                              (pw, half, win, r1, r2) so each pair block holds
                              both channel halves contiguously
  at_sh  [16hh, 128, 16384]   attn fp8-e3m4, per-(win,nh,q)-row scaled on host
                              (amax -> 15.0); partition = 64*win+k, free =
                              (pair, nh, q); dequant scales applied in decode
  w_prep [256c, 256o]         (w_conv * inv_std[:,None]).T fp16 (BN folded)
  bias   [128, 1024]          (beta - mean*inv_std) broadcast over partitions
  out    [16hh, 128p, 4096]   raw staging dump fp16; host decodes

Per group of GB=4 window-pairs (pixels on psum partitions), software-pipelined
with LOOKAHEAD=2 so the cross-engine chain conv->ADD(DVE)->RELU(ACT)->attn(PE)
hides across two pipeline iterations (keeps the PE fed and the HAM clock gate
warm). Each iteration emits finish(g-2) BEFORE stage(g) so psum evacs are
never stuck behind fresh ADDs in the DVE FIFO:
  finish(g-2):
    attn (PE): per (pair, head): out[32,64] = V[:,32nh:+32].T @ At[:,64nh:+64]
               fp16 stationary (block-diag V) x fp8-e3m4 moving, K=128, N=64,
               tile_position=(0, 32*(nh%4)) -> 4 column-strips packed
    evac (DVE): attn psum -> staging fp16
    store: 0.5 MiB per group via the scalar hwdge ring
  stage(g):
    conv (PE): psum[128pix=(win,r1,r2), 256ch] = x_pair.T @ w_prep
               fp8 stationary x fp16 moving, 2 matmuls (K=128 halves), N=256
    bias (DVE): tv = psum + bias_tile
    relu (ACT): block-diagonal V [128, (nh,win,d)]: diag cells = relu(tv),
                off-diag cells stay zero (zeroed once at start)

Quantization (harness gate rel_err < 2e-2; this kernel measures ~1.6e-2):
attn + x in fp8-e3m4 (4 mantissa bits), attn rows scaled to amax=15 with the
scales folded into the host-side decode; V/w fp16; all matmul accum fp32.
"""

import numpy as np
import ml_dtypes
from contextlib import ExitStack

import concourse.bacc as bacc
import concourse.tile as tile
import concourse.mybir as mybir
from concourse.bass_utils import run_bass_kernel_spmd

F32 = mybir.dt.float32
F16 = mybir.dt.float16
F8E3 = mybir.dt.float8e3
RELU = mybir.ActivationFunctionType.Relu

EPS = 1e-5
NCORES = 8

_cached_nc = None


def _build_program(n_vbd=4, at_bufs=3, LOOKAHEAD=2):
    nc = bacc.Bacc("TRN2", target_bir_lowering=False, debug=False)

    x_d = nc.dram_tensor("x_sh", [128, 16, 4096], F8E3, kind="ExternalInput")
    at_d = nc.dram_tensor("at_sh", [16, 128, 16384], F8E3, kind="ExternalInput")
    wc_d = nc.dram_tensor("w_prep", [256, 256], F16, kind="ExternalInput")
    b_d = nc.dram_tensor("bias", [128, 1024], F32, kind="ExternalInput")
    out_d = nc.dram_tensor("out_sh", [16, 128, 4096], F16, kind="ExternalOutput")

    GB = 4                   # pairs per elementwise batch group

    with tile.TileContext(nc) as tc, ExitStack() as ctx:
        const = ctx.enter_context(tc.tile_pool(name="const", bufs=1))
        xp = ctx.enter_context(tc.tile_pool(name="xp", bufs=3))
        atp = ctx.enter_context(tc.tile_pool(name="atp", bufs=at_bufs))
        vbdp = ctx.enter_context(tc.tile_pool(name="vbdp", bufs=1))
        tvp = ctx.enter_context(tc.tile_pool(name="tvp", bufs=4))
        stp = ctx.enter_context(tc.tile_pool(name="stp", bufs=3))
        pscp = ctx.enter_context(tc.tile_pool(name="pscp", bufs=2, space="PSUM"))
        psap = ctx.enter_context(tc.tile_pool(name="psap", bufs=2, space="PSUM"))

        # const loads go on the scalar HWDGE ring so the sync ring's FIFO
        # starts with the bulk at/x loads immediately
        w0 = const.tile([128, 256], F16, name="w0")
        w1 = const.tile([128, 256], F16, name="w1")
        nc.scalar.dma_start(out=w0, in_=wc_d[0:128, :])
        nc.scalar.dma_start(out=w1, in_=wc_d[128:256, :])
        bias = const.tile([128, 1024], F32, name="bias_t")
        nc.scalar.dma_start(out=bias, in_=b_d[:, :])

        # HAM warm-up: the PE clock gate defaults to 1.2 GHz and releases to
        # 2.4 GHz only after ~3.4us of sustained matmul activity. The PE
        # would otherwise idle ~8us waiting for the first x tile, so fill
        # that window with dummy matmuls that read only w0 (already loaded
        # via the scalar ring at ~3us) -- no extra memset, no new deps.
        wps = psap.tile([128, 256], F32, tag="pa4", name="warm_ps")
        for i in range(28):
            nc.tensor.matmul(wps, w0[:, 0:128], w0, start=True, stop=True)

        # Block-diagonal V tiles for GB pairs each: columns =
        # (pair GB, nh 16, win 2, d 16). Zeroed once; the relu writes only the
        # diagonal cells (win0 -> rows 0:64 of win-0 columns, win1 -> rows
        # 64:128 of win-1 columns), so the zeros persist across reuse and each
        # V[:, 512p+32nh:+32] is exactly block-diag(V0, V1).
        # lazily zeroed: only the first two upfront so the DVE queue reaches
        # the first ADD quickly; the rest are zeroed one per iteration below
        vbd = [vbdp.tile([128, 512 * GB], F16, tag=f"vbd{i}", name=f"vbd{i}")
               for i in range(n_vbd)]
        nc.vector.memset(vbd[0], 0.0)
        nc.vector.memset(vbd[1], 0.0)
        vbd_zeroed = 2

        # 3-stage software pipeline with LOOKAHEAD=2: for group g issue
        # conv(g)+add(g)+relu(g), then finish(g-2) = attn+evac+store. The
        # cross-engine latency chain conv->ADD(DVE)->RELU(ACT)->attn(PE)
        # (~3.5us) then hides across two pipeline cycles, so the PE never
        # starves (which would also re-throttle the HAM clock gate).
        pending = []
        vbd_i = 0
        evac_i = 0

        def iteration(cur, prev):
            """Emit one pipeline iteration: conv+add+relu for `cur`, attn+
            evac+store for `prev` (LOOKAHEAD iterations older), with the conv
            pair-blocks INTERLEAVED between attn pair-blocks. Each ~110ns
            conv matmul lets the sequencer rebuild PE-queue lead so the attn
            matmuls run queue-fed (multiple column-strips streaming) instead
            of dispatch-dribble."""
            nonlocal vbd_i, evac_i, vbd_zeroed
            ps4 = tv4 = V4c = None
            if prev is not None:
                V4, at_p, st_p, hh_p, bg_p = prev
                pa4 = psap.tile([128, 256 * GB], F32, tag="pa4",
                                name=f"pa4_{hh_p}_{bg_p}")
                for p in range(GB):
                    ploc = GB * bg_p + p       # pair index in at tile
                    for j in range(4):
                        for quad in range(4):
                            nh = 4 * j + quad
                            nc.tensor.matmul(
                                pa4[32 * quad:32 * quad + 32,
                                    256 * p + 64 * j:256 * p + 64 * j + 64],
                                V4[:, 512 * p + 32 * nh:
                                   512 * p + 32 * nh + 32],
                                at_p[:, 1024 * ploc + 64 * nh:
                                     1024 * ploc + 64 * nh + 64],
                                start=True, stop=True,
                                tile_position=(0, 32 * quad))
                osl_p = slice(1024 * bg_p, 1024 * bg_p + 1024)
                nc.vector.tensor_copy(st_p[:, osl_p], pa4)
                evac_i += 1
                nc.scalar.dma_start(out=out_d[hh_p, :, osl_p],
                                    in_=st_p[:, osl_p])
            if cur is not None:
                xt, at, st, hh, bg = cur
                if vbd_zeroed < n_vbd:
                    nc.vector.memset(vbd[vbd_zeroed], 0.0)
                    vbd_zeroed += 1
                ps4 = pscp.tile([128, 256 * GB], F32, tag="ps4",
                                name=f"ps4_{hh}_{bg}")
                for p in range(GB):
                    p16 = GB * bg + p          # pair index in hh
                    xsl0 = slice(256 * p16, 256 * p16 + 128)
                    xsl1 = slice(256 * p16 + 128, 256 * p16 + 256)
                    osl = slice(256 * p, 256 * p + 256)
                    nc.tensor.matmul(ps4[:, osl], xt[:, xsl0], w0,
                                     start=True, stop=False)
                    nc.tensor.matmul(ps4[:, osl], xt[:, xsl1], w1,
                                     start=False, stop=True)
                tv4 = tvp.tile([128, 256 * GB], F16, tag="tv4",
                               name=f"tv4_{hh}_{bg}")
                nc.vector.tensor_add(tv4, ps4, bias)
                V4c = vbd[vbd_i % n_vbd]
                vbd_i += 1
                Vr = V4c.rearrange("pt (p nh two d) -> pt p nh two d",
                                   p=GB, nh=16, two=2, d=16)
                tvr = tv4.rearrange("pt (p a b) -> pt p a b", p=GB, a=16)
                nc.scalar.activation(Vr[0:64, :, :, 0, :], tvr[0:64], RELU)
                nc.scalar.activation(Vr[64:128, :, :, 1, :], tvr[64:128],
                                     RELU)
                return (V4c, at, st, hh, bg)
            return None

        for hh in range(16):
            xt = xp.tile([128, 4096], F8E3, tag="xt", name=f"xt_{hh}")
            if hh == 0:
                # pair-major x layout: quarter q holds exactly group q's
                # pairs, so conv(0,0) can start after the first 256 KiB
                for q in range(4):
                    nc.sync.dma_start(out=xt[:, 1024 * q:1024 * q + 1024],
                                      in_=x_d[:, hh, 1024 * q:1024 * q + 1024])
            else:
                nc.sync.dma_start(out=xt, in_=x_d[:, hh, :])
            # all 16 heads arrive fp8-e3m4 (per-row scaled on host; dequant
            # on host during decode); the PE consumes fp8 moving operands
            # directly -- no widen pass. Two half-loads so the first groups
            # of the row do not wait on the full 2 MiB tile.
            at = atp.tile([128, 16384], F8E3, tag="at", name=f"at_{hh}")
            nc.sync.dma_start(out=at[:, 0:8192], in_=at_d[hh, :, 0:8192])
            nc.sync.dma_start(out=at[:, 8192:16384],
                              in_=at_d[hh, :, 8192:16384])
            st = stp.tile([128, 4096], F16, tag="st", name=f"st_{hh}")

            for bg in range(4):
                prev = pending.pop(0) if len(pending) >= LOOKAHEAD else None
                pending.append(iteration((xt, at, st, hh, bg), prev))
        for state in pending:
            iteration(None, state)

    nc.compile()
    return nc


def _shard_inputs(x, attn_i, w_conv, bn_gamma, bn_beta, bn_mean, bn_var):
    inv_std = (bn_gamma / np.sqrt(bn_var + np.float32(EPS))).astype(np.float32)
    shift = (bn_beta - bn_mean * inv_std).astype(np.float32)
    bias_tile = np.ascontiguousarray(
        np.broadcast_to(np.tile(shift, 4)[None, :], (128, 1024))
    ).astype(np.float32)
    w_prep = np.ascontiguousarray(
        (w_conv * inv_std[:, None]).T).astype(np.float16)
    x16 = x.astype(np.float32)
    # all 16 heads: fp8 e3m4 with per-(win,head,q)-row scales mapping the
    # row amax to 15.0 (e3m4 max normal is 15.5); dequant on host in decode
    amax = np.maximum(np.abs(attn_i).max(axis=3, keepdims=True), 1e-9)
    s_a = (amax / np.float32(15.0)).astype(np.float32)  # [4096, 16, 64, 1]
    a8 = (attn_i / s_a).astype(ml_dtypes.float8_e3m4)
    in_maps = []
    scales = []
    for core in range(NCORES):
        b, rh = core // 2, core % 2
        x_sh = x16[b, :, 128 * rh:128 * rh + 128, :]
        # pair-major layout: [cl, hh, (pw, half, win, r1, r2)] -- each pair's
        # 256-col block holds both channel halves, so a quarter of the row
        # covers one whole conv group
        x_sh = np.ascontiguousarray(
            x_sh.reshape(2, 128, 16, 8, 16, 2, 8).transpose(1, 2, 4, 0, 5, 3, 6)
        ).reshape(128, 16, 4096).astype(ml_dtypes.float8_e3m4)
        sl = slice(1024 * b + 512 * rh, 1024 * b + 512 * rh + 512)

        def prep(a):  # [512, 16, 64, 64] -> [16, 128, 16384], pair-transposed
            p = a.reshape(256, 2, 16, 64, 64).transpose(0, 1, 4, 2, 3) \
                .reshape(16, 16, 128, 1024)
            return np.ascontiguousarray(
                p.transpose(0, 2, 1, 3)).reshape(16, 128, 16384)

        scales.append(s_a[sl])
        in_maps.append(dict(x_sh=x_sh, at_sh=prep(a8[sl]),
                            w_prep=w_prep, bias=bias_tile))
    return in_maps, scales


def _unshard_output(results, scales):
    out = np.empty((4, 256, 256, 256), np.float32)
    for core in range(NCORES):
        b, rh = core // 2, core % 2
        raw = np.asarray(results[core]["out_sh"], np.float32)  # [16, 128, 4096]
        # partition = (quad4, win2, d16); f = pw*256 + j*64 + ws1*8 + ws2
        r = raw.reshape(16, 4, 2, 16, 16, 4, 8, 8)  # hh,quad,win,d,pw,j,ws1,ws2
        # dequant all heads: scales[core] [512=(hh,pw,win), 16nh, 64q, 1]
        s = scales[core].reshape(16, 16, 2, 4, 4, 8, 8)  # hh,pw,win,j,quad,ws1,ws2
        # -> [hh, quad, win, 1(d), pw, j, ws1, ws2]
        s = s.transpose(0, 4, 2, 1, 3, 5, 6)[:, :, :, None]
        r = r * s
        # ch = 16*(4j+quad)+d ; h = 8hh+ws1 ; w = 16pw+8win+ws2
        oc = r.transpose(5, 1, 3, 0, 6, 4, 2, 7).reshape(256, 128, 256)
        out[b, :, 128 * rh:128 * rh + 128, :] = oc
    return out


def get_program():
    global _cached_nc
    if _cached_nc is None:
        _cached_nc = _build_program()
    return _cached_nc


def run_sharded(in_maps, trace=False, **kwargs):
    nc = get_program()
    return run_bass_kernel_spmd(nc, in_maps, list(range(NCORES)),
                                trace=trace, **kwargs)


def kernel(x, attn_i, w_conv, bn_gamma, bn_beta, bn_mean, bn_var):
    x = np.asarray(x, dtype=np.float32)
    attn_i = np.asarray(attn_i, dtype=np.float32)
    w_conv = np.asarray(w_conv, dtype=np.float32)
    bn_gamma = np.asarray(bn_gamma, dtype=np.float32)
    bn_beta = np.asarray(bn_beta, dtype=np.float32)
    bn_mean = np.asarray(bn_mean, dtype=np.float32)
    bn_var = np.asarray(bn_var, dtype=np.float32)
    in_maps, scales = _shard_inputs(x, attn_i, w_conv, bn_gamma, bn_beta,
                                    bn_mean, bn_var)
    res = run_sharded(in_maps)
    return _unshard_output(res.results, scales)

